# revision 2
# baseline (speedup 1.0000x reference)
"""Trainium2 Bass kernel for the sliding-window CNN problem.

Computes, for x[B=32, WORDS=512, E=256], W[1024, 1280], b[1024]:
    z[b,t,h] = sum_{w<5, e<256} x[b, t+w, e] * W[h, w*256+e]   (T = 508 windows)
    out[b,h] = relu(max_t z[b,t,h] + b[h])

Strategy: data-parallel over batch (4 batches per core, 8 cores).
Per core the window conv is 10 accumulating matmuls (5 window shifts x 2
feature chunks of 128) per [128h x 508t] PSUM tile; the window shift is a
free SBUF column offset on the moving operand.  Max over time on DVE,
bias+relu fused on ScalarE, one contiguous DMA out.
"""

import numpy as np
import ml_dtypes

import concourse.bacc as bacc
import concourse.mybir as mybir
import concourse.tile as tile
from concourse.bass_utils import run_bass_kernel_spmd

B, WORDS, E = 32, 512, 256
WIN = 5
HIDDEN = 1024
T = WORDS - WIN + 1          # 508 sliding windows
NCORES = 8
BPC = B // NCORES            # 4 batches per core
F = WIN * E                  # 1280 contraction features
KC = F // 128                # 10 contraction chunks
HC = HIDDEN // 128           # 8 hidden chunks
EC = E // 128                # 2 feature chunks per window position

BF16 = mybir.dt.bfloat16
FP32 = mybir.dt.float32

_CACHE = {}


def _build():
    nc = bacc.Bacc(None, target_bir_lowering=False)
    xT = nc.dram_tensor("xT", [BPC, EC, 128, WORDS], BF16, kind="ExternalInput")
    wT = nc.dram_tensor("wT", [KC, 128, HIDDEN], BF16, kind="ExternalInput")
    bias = nc.dram_tensor("bias", [128, HC], FP32, kind="ExternalInput")
    # out[p, c] with c = b*HC + hc holds result for hidden unit hc*128+p, batch b
    out = nc.dram_tensor("out", [128, BPC * HC], FP32, kind="ExternalOutput")

    with tile.TileContext(nc) as tc:
        with (
            tc.tile_pool(name="xin", bufs=1) as xpool,
            tc.tile_pool(name="wgt", bufs=1) as wpool,
            tc.tile_pool(name="ps", bufs=8, space="PSUM") as pspool,
            tc.tile_pool(name="post", bufs=4) as postpool,
            tc.tile_pool(name="cst", bufs=1) as cstpool,
        ):
            xt = {}
            for b in range(BPC):
                for ec in range(EC):
                    t = xpool.tile([128, WORDS], BF16, tag=f"x_{b}_{ec}")
                    nc.sync.dma_start(t[:], xT[b, ec])
                    xt[(b, ec)] = t
            wt = []
            for kc in range(KC):
                t = wpool.tile([128, HIDDEN], BF16, tag=f"w_{kc}")
                nc.sync.dma_start(t[:], wT[kc])
                wt.append(t)
            bias_sb = cstpool.tile([128, HC], FP32, tag="bias")
            nc.sync.dma_start(bias_sb[:], bias[:])
            res = cstpool.tile([128, BPC * HC], FP32, tag="res")

            for b in range(BPC):
                for hc in range(HC):
                    ps = pspool.tile([128, T], FP32, tag="ps")
                    for kc in range(KC):
                        w, ec = divmod(kc, EC)
                        nc.tensor.matmul(
                            ps[:],
                            wt[kc][:, hc * 128:(hc + 1) * 128],
                            xt[(b, ec)][:, w:w + T],
                            start=(kc == 0),
                            stop=(kc == KC - 1),
                        )
                    mx = postpool.tile([128, 1], FP32, tag="mx")
                    nc.vector.reduce_max(mx[:], ps[:], axis=mybir.AxisListType.X)
                    c = b * HC + hc
                    nc.scalar.activation(
                        res[:, c:c + 1], mx[:],
                        mybir.ActivationFunctionType.Relu,
                        bias=bias_sb[:, hc:hc + 1],
                    )
            nc.sync.dma_start(out[:, :], res[:])
    nc.finalize()
    return nc


def _prep(input, W, b):
    # x[b, t, e] -> xT[b, ec, p, t] = x[b, t, ec*128+p], bf16
    xT = np.ascontiguousarray(np.transpose(input, (0, 2, 1))).reshape(
        B, EC, 128, WORDS).astype(ml_dtypes.bfloat16)
    # W[h, f] -> wT[kc, p, h] = W[h, kc*128+p], bf16
    wT = np.ascontiguousarray(W.T).reshape(KC, 128, HIDDEN).astype(ml_dtypes.bfloat16)
    # b[h] -> bias[p, hc] = b[hc*128+p], fp32
    bias = np.ascontiguousarray(b.reshape(HC, 128).T).astype(np.float32)
    return xT, wT, bias


def run(inputs, trace=False, **kwargs):
    if "nc" not in _CACHE:
        _CACHE["nc"] = _build()
    nc = _CACHE["nc"]
    xT, wT, bias = _prep(inputs["input"], inputs["W"], inputs["b"])
    in_maps = [
        {"xT": xT[c * BPC:(c + 1) * BPC], "wT": wT, "bias": bias}
        for c in range(NCORES)
    ]
    res = run_bass_kernel_spmd(nc, in_maps, list(range(NCORES)), trace=trace, **kwargs)
    # out[p, c] with c = b*HC+hc -> full[core*BPC + b, hc*128 + p]
    parts = []
    for c in range(NCORES):
        o = res.results[c]["out"]              # [128, BPC*HC]
        o = o.T.reshape(BPC, HC, 128).reshape(BPC, HIDDEN)
        parts.append(o)
    full = np.concatenate(parts, axis=0).astype(np.float32)
    return full, res


def kernel(**inputs):
    out, _ = run(inputs, trace=False)
    return out


# revision 5
# speedup vs baseline: 1.0401x; 1.0401x over previous
"""Trainium2 Bass kernel for the sliding-window CNN problem.

Computes, for x[B=32, WORDS=512, E=256], W[1024, 1280], b[1024]:
    z[b,t,h] = sum_{w<5, e<256} x[b, t+w, e] * W[h, w*256+e]   (T = 508 windows)
    out[b,h] = relu(max_t z[b,t,h] + b[h])

Strategy: data-parallel over batch (4 batches per core, 8 cores).
Per core the window conv is 10 accumulating matmuls (5 window shifts x 2
feature chunks of 128) per [128h x 508t] PSUM tile; the window shift is a
free SBUF column offset on the moving operand.  fp16 operands (same PE
rate as bf16, ~8x better accuracy), fp32 PSUM accumulation.  Loop is
kc-outer over all 8 hidden chunks (8 PSUM banks in flight) so the PE's
weight-consumption rate stays below the DMA delivery rate and compute
overlaps the weight loads.  Max over time on DVE, bias+relu fused on
ScalarE, per-batch DMA out.
"""

import numpy as np

import concourse.bacc as bacc
import concourse.mybir as mybir
import concourse.tile as tile
from concourse.bass_utils import run_bass_kernel_spmd

B, WORDS, E = 32, 512, 256
WIN = 5
HIDDEN = 1024
T = WORDS - WIN + 1          # 508 sliding windows
NCORES = 8
BPC = B // NCORES            # 4 batches per core
F = WIN * E                  # 1280 contraction features
KC = F // 128                # 10 contraction chunks
HC = HIDDEN // 128           # 8 hidden chunks
EC = E // 128                # 2 feature chunks per window position

FP16 = mybir.dt.float16
FP32 = mybir.dt.float32

_CACHE = {}


def _build():
    nc = bacc.Bacc(None, target_bir_lowering=False)
    # xT[p, b, ec, t] = x[b, t, ec*128+p]
    xT = nc.dram_tensor("xT", [128, BPC, EC, WORDS], FP16, kind="ExternalInput")
    # wT[p, kc, h] = W[h, kc*128+p]
    wT = nc.dram_tensor("wT", [128, KC, HIDDEN], FP16, kind="ExternalInput")
    bias = nc.dram_tensor("bias", [128, HC], FP32, kind="ExternalInput")
    # out[b, p, hc] = result for batch b, hidden unit hc*128+p
    out = nc.dram_tensor("out", [BPC, 128, HC], FP32, kind="ExternalOutput")

    with tile.TileContext(nc) as tc:
        with (
            tc.tile_pool(name="xin", bufs=1) as xpool,
            tc.tile_pool(name="wgt", bufs=1) as wpool,
            tc.tile_pool(name="ps", bufs=1, space="PSUM") as pspool,
            tc.tile_pool(name="post", bufs=2) as postpool,
            tc.tile_pool(name="cst", bufs=1) as cstpool,
        ):
            # DMA emission order = need order: batch0 x, bias, all weights,
            # remaining batches.  Every DMA is per-partition contiguous in
            # DRAM thanks to the host-side layouts above.
            xt = []
            t0 = xpool.tile([128, EC * WORDS], FP16, tag="x_0")
            nc.sync.dma_start(t0[:], xT[:, 0])
            xt.append(t0)
            bias_sb = cstpool.tile([128, HC], FP32, tag="bias")
            nc.sync.dma_start(bias_sb[:], bias[:])
            wt = []
            for kc in range(KC):
                t = wpool.tile([128, HIDDEN], FP16, tag=f"w_{kc}")
                nc.sync.dma_start(t[:], wT[:, kc])
                wt.append(t)
            for b in range(1, BPC):
                t = xpool.tile([128, EC * WORDS], FP16, tag=f"x_{b}")
                nc.sync.dma_start(t[:], xT[:, b])
                xt.append(t)

            for b in range(BPC):
                ps = [
                    pspool.tile([128, T], FP32, tag=f"ps{hc}", name=f"ps_{b}_{hc}")
                    for hc in range(HC)
                ]
                for kc in range(KC):
                    w, ec = divmod(kc, EC)
                    rhs = xt[b][:, ec * WORDS + w: ec * WORDS + w + T]
                    for hc in range(HC):
                        nc.tensor.matmul(
                            ps[hc][:],
                            wt[kc][:, hc * 128:(hc + 1) * 128],
                            rhs,
                            start=(kc == 0),
                            stop=(kc == KC - 1),
                        )
                res = postpool.tile([128, HC], FP32, tag="res")
                for hc in range(HC):
                    mx = postpool.tile([128, 1], FP32, tag=f"mx{hc}")
                    nc.vector.reduce_max(mx[:], ps[hc][:], axis=mybir.AxisListType.X)
                    nc.scalar.activation(
                        res[:, hc:hc + 1], mx[:],
                        mybir.ActivationFunctionType.Relu,
                        bias=bias_sb[:, hc:hc + 1],
                    )
                nc.sync.dma_start(out[b], res[:])
    nc.finalize()
    return nc


def _prep(input, W, b):
    x = np.asarray(input, dtype=np.float32)
    # x[b, t, e] -> xT[p, b, ec, t] = x[b, t, ec*128+p]
    y = x.transpose(2, 0, 1).reshape(EC, 128, B, WORDS)      # [ec, p, b, t]
    xT = np.ascontiguousarray(y.transpose(1, 2, 0, 3)).astype(np.float16)  # [p,b,ec,t]
    # W[h, f] -> wT[p, kc, h] = W[h, kc*128+p]
    wt = np.asarray(W, dtype=np.float32).T.reshape(KC, 128, HIDDEN)  # [kc, p, h]
    wT = np.ascontiguousarray(wt.transpose(1, 0, 2)).astype(np.float16)  # [p, kc, h]
    # b[h] -> bias[p, hc] = b[hc*128+p]
    bias = np.ascontiguousarray(np.asarray(b, np.float32).reshape(HC, 128).T)
    return xT, wT, bias


def run(inputs, trace=False, **kwargs):
    if "nc" not in _CACHE:
        _CACHE["nc"] = _build()
    nc = _CACHE["nc"]
    xT, wT, bias = _prep(inputs["input"], inputs["W"], inputs["b"])
    in_maps = [
        {"xT": xT[:, c * BPC:(c + 1) * BPC], "wT": wT, "bias": bias}
        for c in range(NCORES)
    ]
    in_maps = [{k: np.ascontiguousarray(v) for k, v in m.items()} for m in in_maps]
    res = run_bass_kernel_spmd(nc, in_maps, list(range(NCORES)), trace=trace, **kwargs)
    # out[b, p, hc] -> full[core*BPC + b, hc*128 + p]
    parts = []
    for c in range(NCORES):
        o = res.results[c]["out"]              # [BPC, 128, HC]
        parts.append(o.transpose(0, 2, 1).reshape(BPC, HIDDEN))
    full = np.concatenate(parts, axis=0).astype(np.float32)
    return full, res


def kernel(**inputs):
    out, _ = run(inputs, trace=False)
    return out


# revision 6
# speedup vs baseline: 1.0662x; 1.0251x over previous
"""Trainium2 Bass kernel for the sliding-window CNN problem.

Computes, for x[B=32, WORDS=512, E=256], W[1024, 1280], b[1024]:
    z[b,t,h] = sum_{w<5, e<256} x[b, t+w, e] * W[h, w*256+e]   (T = 508 windows)
    out[b,h] = relu(max_t z[b,t,h] + b[h])

Strategy: data-parallel over batch (4 batches per core, 8 cores).
Per core the window conv is 10 accumulating matmuls (5 window shifts x 2
feature chunks of 128) per [128h x 508t] PSUM tile; the window shift is a
free SBUF column offset on the moving operand.  fp16 operands (same PE
rate as bf16, ~8x better accuracy), fp32 PSUM accumulation.  Loop is
kc-outer over all 8 hidden chunks (8 PSUM banks in flight) so the PE's
weight-consumption rate stays below the DMA delivery rate and compute
overlaps the weight loads.  Max over time on DVE, bias+relu fused on
ScalarE, per-batch DMA out.
"""

import numpy as np

import concourse.bacc as bacc
import concourse.mybir as mybir
import concourse.tile as tile
from concourse.bass_utils import run_bass_kernel_spmd

B, WORDS, E = 32, 512, 256
WIN = 5
HIDDEN = 1024
T = WORDS - WIN + 1          # 508 sliding windows
NCORES = 8
BPC = B // NCORES            # 4 batches per core
F = WIN * E                  # 1280 contraction features
KC = F // 128                # 10 contraction chunks
HC = HIDDEN // 128           # 8 hidden chunks
EC = E // 128                # 2 feature chunks per window position

FP16 = mybir.dt.float16
FP32 = mybir.dt.float32

_CACHE = {}


def _build():
    nc = bacc.Bacc(None, target_bir_lowering=False)
    # xT[p, b, ec, t] = x[b, t, ec*128+p]
    xT = nc.dram_tensor("xT", [128, BPC, EC, WORDS], FP16, kind="ExternalInput")
    # wT[p, kc, h] = W[h, kc*128+p]
    wT = nc.dram_tensor("wT", [128, KC, HIDDEN], FP16, kind="ExternalInput")
    bias = nc.dram_tensor("bias", [128, HC], FP32, kind="ExternalInput")
    # out[b, p, hc] = result for batch b, hidden unit hc*128+p
    out = nc.dram_tensor("out", [BPC, 128, HC], FP32, kind="ExternalOutput")

    with tile.TileContext(nc) as tc:
        with (
            tc.tile_pool(name="xin", bufs=1) as xpool,
            tc.tile_pool(name="wgt", bufs=1) as wpool,
            tc.tile_pool(name="ps", bufs=1, space="PSUM") as pspool,
            tc.tile_pool(name="post", bufs=2) as postpool,
            tc.tile_pool(name="cst", bufs=1) as cstpool,
        ):
            # DMA emission order = need order: batch0 x, bias, all weights,
            # remaining batches.  Every DMA is per-partition contiguous in
            # DRAM thanks to the host-side layouts above.
            # tiny bias DMA first warms the HWDGE queue; then batch0's x in
            # two halves interleaved with the first weight tiles so the
            # first matmul's inputs land as early as possible.
            bias_sb = cstpool.tile([128, HC], FP32, tag="bias")
            nc.sync.dma_start(bias_sb[:], bias[:])
            xt = [xpool.tile([128, EC * WORDS], FP16, tag="x_0", name="x_0")]
            nc.sync.dma_start(xt[0][:, 0:WORDS], xT[:, 0, 0])
            wt = [wpool.tile([128, HIDDEN], FP16, tag="w_0", name="w_0")]
            nc.sync.dma_start(wt[0][:], wT[:, 0])
            nc.sync.dma_start(xt[0][:, WORDS:2 * WORDS], xT[:, 0, 1])
            for kc in range(1, KC):
                t = wpool.tile([128, HIDDEN], FP16, tag=f"w_{kc}", name=f"w_{kc}")
                nc.sync.dma_start(t[:], wT[:, kc])
                wt.append(t)
            for b in range(1, BPC):
                t = xpool.tile([128, EC * WORDS], FP16, tag=f"x_{b}", name=f"x_{b}")
                nc.sync.dma_start(t[:], xT[:, b])
                xt.append(t)

            def emit_group(b, hc, ps):
                """All KC accumulating matmuls for psum group (b, hc)."""
                for kc in range(KC):
                    w, ec = divmod(kc, EC)
                    nc.tensor.matmul(
                        ps[:],
                        wt[kc][:, hc * 128:(hc + 1) * 128],
                        xt[b][:, ec * WORDS + w: ec * WORDS + w + T],
                        start=(kc == 0),
                        stop=(kc == KC - 1),
                    )

            def emit_post(b, hc, ps, res):
                mx = postpool.tile([128, 1], FP32, tag=f"mx{hc}", name=f"mx_{b}_{hc}")
                nc.vector.reduce_max(mx[:], ps[:], axis=mybir.AxisListType.X)
                nc.scalar.activation(
                    res[:, hc:hc + 1], mx[:],
                    mybir.ActivationFunctionType.Relu,
                    bias=bias_sb[:, hc:hc + 1],
                )

            for b in range(BPC):
                ps = [
                    pspool.tile([128, T], FP32, tag=f"ps{hc}", name=f"ps_{b}_{hc}")
                    for hc in range(HC)
                ]
                res = postpool.tile([128, HC], FP32, tag="res", name=f"res_{b}")
                if b < BPC - 1:
                    # kc-outer: all 8 banks accumulate in parallel; PE's
                    # weight consumption rate stays below DMA delivery, so
                    # compute starts as soon as wt[0] lands.
                    for kc in range(KC):
                        w, ec = divmod(kc, EC)
                        rhs = xt[b][:, ec * WORDS + w: ec * WORDS + w + T]
                        for hc in range(HC):
                            nc.tensor.matmul(
                                ps[hc][:],
                                wt[kc][:, hc * 128:(hc + 1) * 128],
                                rhs,
                                start=(kc == 0),
                                stop=(kc == KC - 1),
                            )
                    for hc in range(HC):
                        emit_post(b, hc, ps[hc], res)
                else:
                    # last batch: hc-outer so groups finish staggered and
                    # the reduce/act chain overlaps the remaining matmuls
                    # instead of serializing after the last one.
                    for hc in range(HC):
                        emit_group(b, hc, ps[hc])
                        emit_post(b, hc, ps[hc], res)
                nc.sync.dma_start(out[b], res[:])
    nc.finalize()
    return nc


def _prep(input, W, b):
    x = np.asarray(input, dtype=np.float32)
    # x[b, t, e] -> xT[p, b, ec, t] = x[b, t, ec*128+p]
    y = x.transpose(2, 0, 1).reshape(EC, 128, B, WORDS)      # [ec, p, b, t]
    xT = np.ascontiguousarray(y.transpose(1, 2, 0, 3)).astype(np.float16)  # [p,b,ec,t]
    # W[h, f] -> wT[p, kc, h] = W[h, kc*128+p]
    wt = np.asarray(W, dtype=np.float32).T.reshape(KC, 128, HIDDEN)  # [kc, p, h]
    wT = np.ascontiguousarray(wt.transpose(1, 0, 2)).astype(np.float16)  # [p, kc, h]
    # b[h] -> bias[p, hc] = b[hc*128+p]
    bias = np.ascontiguousarray(np.asarray(b, np.float32).reshape(HC, 128).T)
    return xT, wT, bias


def run(inputs, trace=False, **kwargs):
    if "nc" not in _CACHE:
        _CACHE["nc"] = _build()
    nc = _CACHE["nc"]
    xT, wT, bias = _prep(inputs["input"], inputs["W"], inputs["b"])
    in_maps = [
        {"xT": xT[:, c * BPC:(c + 1) * BPC], "wT": wT, "bias": bias}
        for c in range(NCORES)
    ]
    in_maps = [{k: np.ascontiguousarray(v) for k, v in m.items()} for m in in_maps]
    res = run_bass_kernel_spmd(nc, in_maps, list(range(NCORES)), trace=trace, **kwargs)
    # out[b, p, hc] -> full[core*BPC + b, hc*128 + p]
    parts = []
    for c in range(NCORES):
        o = res.results[c]["out"]              # [BPC, 128, HC]
        parts.append(o.transpose(0, 2, 1).reshape(BPC, HIDDEN))
    full = np.concatenate(parts, axis=0).astype(np.float32)
    return full, res


def kernel(**inputs):
    out, _ = run(inputs, trace=False)
    return out


# revision 8
# speedup vs baseline: 1.0710x; 1.0045x over previous
"""Trainium2 Bass kernel for the sliding-window CNN problem.

Computes, for x[B=32, WORDS=512, E=256], W[1024, 1280], b[1024]:
    z[b,t,h] = sum_{w<5, e<256} x[b, t+w, e] * W[h, w*256+e]   (T = 508 windows)
    out[b,h] = relu(max_t z[b,t,h] + b[h])

Strategy: data-parallel over batch (4 batches per core, 8 cores).
Per core the window conv is 10 accumulating matmuls (5 window shifts x 2
feature chunks of 128) per [128h x 508t] PSUM tile; the window shift is a
free SBUF column offset on the moving operand.  fp16 operands (same PE
rate as bf16, ~8x better accuracy), fp32 PSUM accumulation.  Loop is
kc-outer over all 8 hidden chunks (8 PSUM banks in flight) so the PE's
weight-consumption rate stays below the DMA delivery rate and compute
overlaps the weight loads.  Max over time on DVE, bias+relu fused on
ScalarE, per-batch DMA out.
"""

import numpy as np

import concourse.bacc as bacc
import concourse.mybir as mybir
import concourse.tile as tile
from concourse.bass_utils import run_bass_kernel_spmd

B, WORDS, E = 32, 512, 256
WIN = 5
HIDDEN = 1024
T = WORDS - WIN + 1          # 508 sliding windows
NCORES = 8
BPC = B // NCORES            # 4 batches per core
F = WIN * E                  # 1280 contraction features
KC = F // 128                # 10 contraction chunks
HC = HIDDEN // 128           # 8 hidden chunks
EC = E // 128                # 2 feature chunks per window position

FP16 = mybir.dt.float16
FP32 = mybir.dt.float32

_CACHE = {}


def _build():
    nc = bacc.Bacc(None, target_bir_lowering=False)
    # xT[p, b, ec, t] = x[b, t, ec*128+p]
    xT = nc.dram_tensor("xT", [128, BPC, EC, WORDS], FP16, kind="ExternalInput")
    # wT[p, kc, h] = W[h, kc*128+p]
    wT = nc.dram_tensor("wT", [128, KC, HIDDEN], FP16, kind="ExternalInput")
    bias = nc.dram_tensor("bias", [128, HC], FP32, kind="ExternalInput")
    # out[b, p, hc] = result for batch b, hidden unit hc*128+p
    out = nc.dram_tensor("out", [BPC, 128, HC], FP32, kind="ExternalOutput")

    with tile.TileContext(nc) as tc:
        with (
            tc.tile_pool(name="xin", bufs=1) as xpool,
            tc.tile_pool(name="wgt", bufs=1) as wpool,
            tc.tile_pool(name="ps", bufs=1, space="PSUM") as pspool,
            tc.tile_pool(name="post", bufs=2) as postpool,
            tc.tile_pool(name="cst", bufs=1) as cstpool,
        ):
            # DMA emission order = need order: batch0 x, bias, all weights,
            # remaining batches.  Every DMA is per-partition contiguous in
            # DRAM thanks to the host-side layouts above.
            # The first matmul needs x[b0] (first half) + wt[0][:, :hc0] only.
            # Issue those two on separate engine queues (Sync / Scalar) so
            # their HWDGE rings start in parallel; everything else follows
            # on Sync well ahead of the PE's consumption rate.
            xt = [xpool.tile([128, EC * WORDS], FP16, tag="x_0", name="x_0")]
            nc.sync.dma_start(xt[0][:, 0:WORDS], xT[:, 0, 0])
            wt = [wpool.tile([128, HIDDEN], FP16, tag="w_0", name="w_0")]
            nc.scalar.dma_start(wt[0][:, 0:512], wT[:, 0, 0:512])
            nc.scalar.dma_start(wt[0][:, 512:HIDDEN], wT[:, 0, 512:HIDDEN])
            nc.sync.dma_start(xt[0][:, WORDS:2 * WORDS], xT[:, 0, 1])
            for kc in range(1, KC):
                t = wpool.tile([128, HIDDEN], FP16, tag=f"w_{kc}", name=f"w_{kc}")
                nc.sync.dma_start(t[:], wT[:, kc])
                wt.append(t)
            bias_sb = cstpool.tile([128, HC], FP32, tag="bias")
            nc.sync.dma_start(bias_sb[:], bias[:])
            for b in range(1, BPC):
                t = xpool.tile([128, EC * WORDS], FP16, tag=f"x_{b}", name=f"x_{b}")
                nc.sync.dma_start(t[:], xT[:, b])
                xt.append(t)

            def emit_group(b, hc, ps):
                """All KC accumulating matmuls for psum group (b, hc)."""
                for kc in range(KC):
                    w, ec = divmod(kc, EC)
                    nc.tensor.matmul(
                        ps[:],
                        wt[kc][:, hc * 128:(hc + 1) * 128],
                        xt[b][:, ec * WORDS + w: ec * WORDS + w + T],
                        start=(kc == 0),
                        stop=(kc == KC - 1),
                    )

            def emit_post(b, hc, ps, res):
                mx = postpool.tile([128, 1], FP32, tag=f"mx{hc}", name=f"mx_{b}_{hc}")
                nc.vector.reduce_max(mx[:], ps[:], axis=mybir.AxisListType.X)
                nc.scalar.activation(
                    res[:, hc:hc + 1], mx[:],
                    mybir.ActivationFunctionType.Relu,
                    bias=bias_sb[:, hc:hc + 1],
                )

            for b in range(BPC):
                ps = [
                    pspool.tile([128, T], FP32, tag=f"ps{hc}", name=f"ps_{b}_{hc}")
                    for hc in range(HC)
                ]
                res = postpool.tile([128, HC], FP32, tag="res", name=f"res_{b}")
                if b < BPC - 1:
                    # kc-outer: all 8 banks accumulate in parallel; PE's
                    # weight consumption rate stays below DMA delivery, so
                    # compute starts as soon as wt[0] lands.
                    for kc in range(KC):
                        w, ec = divmod(kc, EC)
                        rhs = xt[b][:, ec * WORDS + w: ec * WORDS + w + T]
                        for hc in range(HC):
                            nc.tensor.matmul(
                                ps[hc][:],
                                wt[kc][:, hc * 128:(hc + 1) * 128],
                                rhs,
                                start=(kc == 0),
                                stop=(kc == KC - 1),
                            )
                    for hc in range(HC):
                        emit_post(b, hc, ps[hc], res)
                else:
                    # last batch: hc-outer so groups finish staggered and
                    # the reduce/act chain overlaps the remaining matmuls
                    # instead of serializing after the last one.  Ship the
                    # first half of the results while hc 4-7 still compute.
                    for hc in range(HC):
                        emit_group(b, hc, ps[hc])
                        emit_post(b, hc, ps[hc], res)
                        if hc == 3:
                            nc.sync.dma_start(out[b, :, 0:4], res[:, 0:4])
                    nc.sync.dma_start(out[b, :, 4:HC], res[:, 4:HC])
                    continue
                nc.sync.dma_start(out[b], res[:])
    nc.finalize()
    return nc


def _prep(input, W, b):
    x = np.asarray(input, dtype=np.float32)
    # x[b, t, e] -> xT[p, b, ec, t] = x[b, t, ec*128+p]
    y = x.transpose(2, 0, 1).reshape(EC, 128, B, WORDS)      # [ec, p, b, t]
    xT = np.ascontiguousarray(y.transpose(1, 2, 0, 3)).astype(np.float16)  # [p,b,ec,t]
    # W[h, f] -> wT[p, kc, h] = W[h, kc*128+p]
    wt = np.asarray(W, dtype=np.float32).T.reshape(KC, 128, HIDDEN)  # [kc, p, h]
    wT = np.ascontiguousarray(wt.transpose(1, 0, 2)).astype(np.float16)  # [p, kc, h]
    # b[h] -> bias[p, hc] = b[hc*128+p]
    bias = np.ascontiguousarray(np.asarray(b, np.float32).reshape(HC, 128).T)
    return xT, wT, bias


def run(inputs, trace=False, **kwargs):
    if "nc" not in _CACHE:
        _CACHE["nc"] = _build()
    nc = _CACHE["nc"]
    xT, wT, bias = _prep(inputs["input"], inputs["W"], inputs["b"])
    in_maps = [
        {"xT": xT[:, c * BPC:(c + 1) * BPC], "wT": wT, "bias": bias}
        for c in range(NCORES)
    ]
    in_maps = [{k: np.ascontiguousarray(v) for k, v in m.items()} for m in in_maps]
    res = run_bass_kernel_spmd(nc, in_maps, list(range(NCORES)), trace=trace, **kwargs)
    # out[b, p, hc] -> full[core*BPC + b, hc*128 + p]
    parts = []
    for c in range(NCORES):
        o = res.results[c]["out"]              # [BPC, 128, HC]
        parts.append(o.transpose(0, 2, 1).reshape(BPC, HIDDEN))
    full = np.concatenate(parts, axis=0).astype(np.float32)
    return full, res


def kernel(**inputs):
    out, _ = run(inputs, trace=False)
    return out


# revision 10
# speedup vs baseline: 1.1065x; 1.0332x over previous
"""Trainium2 Bass kernel for the sliding-window CNN problem.

Computes, for x[B=32, WORDS=512, E=256], W[1024, 1280], b[1024]:
    z[b,t,h] = sum_{w<5, e<256} x[b, t+w, e] * W[h, w*256+e]   (T = 508 windows)
    out[b,h] = relu(max_t z[b,t,h] + b[h])

Strategy: data-parallel over batch (4 batches per core, 8 cores).
Per core the window conv is 10 accumulating matmuls (5 window shifts x 2
feature chunks of 128) per [128h x 508t] PSUM tile; the window shift is a
free SBUF column offset on the moving operand.  fp16 operands (same PE
rate as bf16, ~8x better accuracy), fp32 PSUM accumulation.  Loop is
kc-outer over all 8 hidden chunks (8 PSUM banks in flight) so the PE's
weight-consumption rate stays below the DMA delivery rate and compute
overlaps the weight loads.  Max over time on DVE, bias+relu fused on
ScalarE, per-batch DMA out.
"""

import numpy as np

import concourse.bacc as bacc
import concourse.mybir as mybir
import concourse.tile as tile
from concourse.bass_utils import run_bass_kernel_spmd

B, WORDS, E = 32, 512, 256
WIN = 5
HIDDEN = 1024
T = WORDS - WIN + 1          # 508 sliding windows
NCORES = 8
BPC = B // NCORES            # 4 batches per core
F = WIN * E                  # 1280 contraction features
KC = F // 128                # 10 contraction chunks
HC = HIDDEN // 128           # 8 hidden chunks
EC = E // 128                # 2 feature chunks per window position

FP16 = mybir.dt.float16
FP32 = mybir.dt.float32

_CACHE = {}


def _build():
    nc = bacc.Bacc(None, target_bir_lowering=False)
    # xT[p, b, ec, t] = x[b, t, ec*128+p]
    xT = nc.dram_tensor("xT", [128, BPC, EC, WORDS], FP16, kind="ExternalInput")
    # wT[p, kc, h] = W[h, kc*128+p]
    wT = nc.dram_tensor("wT", [128, KC, HIDDEN], FP16, kind="ExternalInput")
    bias = nc.dram_tensor("bias", [128, HC], FP32, kind="ExternalInput")
    # out[b, p, hc] = result for batch b, hidden unit hc*128+p
    out = nc.dram_tensor("out", [BPC, 128, HC], FP32, kind="ExternalOutput")

    with tile.TileContext(nc) as tc:
        with (
            tc.tile_pool(name="xin", bufs=1) as xpool,
            tc.tile_pool(name="wgt", bufs=1) as wpool,
            tc.tile_pool(name="ps", bufs=1, space="PSUM") as pspool,
            tc.tile_pool(name="post", bufs=2) as postpool,
            tc.tile_pool(name="cst", bufs=1) as cstpool,
        ):
            # DMA emission order = need order: batch0 x, bias, all weights,
            # remaining batches.  Every DMA is per-partition contiguous in
            # DRAM thanks to the host-side layouts above.
            # The first matmul needs x[b0] (first half) + wt[0][:, :hc0] only.
            # Issue those two on separate engine queues (Sync / Scalar) so
            # their HWDGE rings start in parallel; everything else follows
            # on Sync well ahead of the PE's consumption rate.
            xt = [xpool.tile([128, EC * WORDS], FP16, tag="x_0", name="x_0")]
            nc.sync.dma_start(xt[0][:, 0:WORDS], xT[:, 0, 0])
            wt = [wpool.tile([128, HIDDEN], FP16, tag="w_0", name="w_0")]
            nc.scalar.dma_start(wt[0][:, 0:512], wT[:, 0, 0:512])
            nc.sync.dma_start(wt[0][:, 512:HIDDEN], wT[:, 0, 512:HIDDEN])
            nc.sync.dma_start(xt[0][:, WORDS:2 * WORDS], xT[:, 0, 1])
            for kc in range(1, KC):
                t = wpool.tile([128, HIDDEN], FP16, tag=f"w_{kc}", name=f"w_{kc}")
                nc.sync.dma_start(t[:], wT[:, kc])
                wt.append(t)
            bias_sb = cstpool.tile([128, HC], FP32, tag="bias")
            nc.sync.dma_start(bias_sb[:], bias[:])
            for b in range(1, BPC):
                t = xpool.tile([128, EC * WORDS], FP16, tag=f"x_{b}", name=f"x_{b}")
                nc.sync.dma_start(t[:], xT[:, b])
                xt.append(t)

            # PE pre-warm: the HAM clock gate holds the PE at 1.2 GHz until
            # it has seen ~3.4us of sustained activity.  While the first
            # input DMAs stream in, run junk matmuls (tiny N, memset
            # operands, dedicated PSUM bank that the real groups only need
            # later) so the real matmul stream starts at 2.4 GHz.
            junk = cstpool.tile([128, 128], FP16, tag="junk")
            nc.gpsimd.memset(junk[:], 0.0)
            ps_junk = pspool.tile([128, 64], FP32, tag="ps7", name="ps_junk")
            for _ in range(50):
                nc.tensor.matmul(
                    ps_junk[:], junk[:], junk[:, 0:64], start=True, stop=True
                )

            def emit_group(b, hc, ps):
                """All KC accumulating matmuls for psum group (b, hc)."""
                for kc in range(KC):
                    w, ec = divmod(kc, EC)
                    nc.tensor.matmul(
                        ps[:],
                        wt[kc][:, hc * 128:(hc + 1) * 128],
                        xt[b][:, ec * WORDS + w: ec * WORDS + w + T],
                        start=(kc == 0),
                        stop=(kc == KC - 1),
                    )

            def emit_post(b, hc, ps, res):
                mx = postpool.tile([128, 1], FP32, tag=f"mx{hc}", name=f"mx_{b}_{hc}")
                nc.vector.reduce_max(mx[:], ps[:], axis=mybir.AxisListType.X)
                nc.scalar.activation(
                    res[:, hc:hc + 1], mx[:],
                    mybir.ActivationFunctionType.Relu,
                    bias=bias_sb[:, hc:hc + 1],
                )

            for b in range(BPC):
                ps = [
                    pspool.tile([128, T], FP32, tag=f"ps{hc}", name=f"ps_{b}_{hc}")
                    for hc in range(HC)
                ]
                res = postpool.tile([128, HC], FP32, tag="res", name=f"res_{b}")
                if b < BPC - 1:
                    # kc-outer: all 8 banks accumulate in parallel; PE's
                    # weight consumption rate stays below DMA delivery, so
                    # compute starts as soon as wt[0] lands.
                    for kc in range(KC):
                        w, ec = divmod(kc, EC)
                        rhs = xt[b][:, ec * WORDS + w: ec * WORDS + w + T]
                        for hc in range(HC):
                            nc.tensor.matmul(
                                ps[hc][:],
                                wt[kc][:, hc * 128:(hc + 1) * 128],
                                rhs,
                                start=(kc == 0),
                                stop=(kc == KC - 1),
                            )
                    for hc in range(HC):
                        emit_post(b, hc, ps[hc], res)
                else:
                    # last batch: hc-outer so groups finish staggered and
                    # the reduce/act chain overlaps the remaining matmuls
                    # instead of serializing after the last one.  Ship the
                    # first half of the results while hc 4-7 still compute.
                    for hc in range(HC):
                        emit_group(b, hc, ps[hc])
                        emit_post(b, hc, ps[hc], res)
                        if hc == 3:
                            nc.sync.dma_start(out[b, :, 0:4], res[:, 0:4])
                    nc.sync.dma_start(out[b, :, 4:HC], res[:, 4:HC])
                    continue
                nc.sync.dma_start(out[b], res[:])
    nc.finalize()
    return nc


def _prep(input, W, b):
    x = np.asarray(input, dtype=np.float32)
    # x[b, t, e] -> xT[p, b, ec, t] = x[b, t, ec*128+p]
    y = x.transpose(2, 0, 1).reshape(EC, 128, B, WORDS)      # [ec, p, b, t]
    xT = np.ascontiguousarray(y.transpose(1, 2, 0, 3)).astype(np.float16)  # [p,b,ec,t]
    # W[h, f] -> wT[p, kc, h] = W[h, kc*128+p]
    wt = np.asarray(W, dtype=np.float32).T.reshape(KC, 128, HIDDEN)  # [kc, p, h]
    wT = np.ascontiguousarray(wt.transpose(1, 0, 2)).astype(np.float16)  # [p, kc, h]
    # b[h] -> bias[p, hc] = b[hc*128+p]
    bias = np.ascontiguousarray(np.asarray(b, np.float32).reshape(HC, 128).T)
    return xT, wT, bias


def run(inputs, trace=False, **kwargs):
    if "nc" not in _CACHE:
        _CACHE["nc"] = _build()
    nc = _CACHE["nc"]
    xT, wT, bias = _prep(inputs["input"], inputs["W"], inputs["b"])
    in_maps = [
        {"xT": xT[:, c * BPC:(c + 1) * BPC], "wT": wT, "bias": bias}
        for c in range(NCORES)
    ]
    in_maps = [{k: np.ascontiguousarray(v) for k, v in m.items()} for m in in_maps]
    res = run_bass_kernel_spmd(nc, in_maps, list(range(NCORES)), trace=trace, **kwargs)
    # out[b, p, hc] -> full[core*BPC + b, hc*128 + p]
    parts = []
    for c in range(NCORES):
        o = res.results[c]["out"]              # [BPC, 128, HC]
        parts.append(o.transpose(0, 2, 1).reshape(BPC, HIDDEN))
    full = np.concatenate(parts, axis=0).astype(np.float32)
    return full, res


def kernel(**inputs):
    out, _ = run(inputs, trace=False)
    return out
